# revision 2
# baseline (speedup 1.0000x reference)
"""Multi-head attention block (B=2, N=2048, C=1024, H=16, D=64) on 8 TRN2 cores.

Sharding: core c -> batch b = c // 4, head-group hg = c % 4 (4 heads per core).
v2: qkv projections via fp8e4 DoubleRow matmuls (0.5 cyc/row) with host-side
residual splits x ~= xh + xl, W*32 ~= Wh + Wl; three cross products
(Wh.xh + Wh.xl + Wl.xh) recover ~bf16 accuracy (measured 1.7e-3 end-to-end
vs 3.7e-3 for the bf16 baseline) at 3/4 of half the bf16 cycle cost.
Scale bookkeeping: q,k carry x32 each -> exp scale 2^-13 folds the 1/8
attention scale; v carries x32 -> Wproj pre-divided by 32 on host.

Each core:
  qkT,V = W_hg @ x_b^T          (fp8 DoubleRow, 12 DR matmuls per psum tile)
  S^T  = kT^T q-chunks          (fp32r, K=64, two heads row-packed per pass)
  P^T  = exp(S^T * 2^-13)       (ScalarE)
  O^T|Z = [V|1]^T @ P^T         (PSUM-accumulated over key tiles)
  O^T /= Z                      (DVE reciprocal; 1/Z row broadcast via DMA)
  y_partial = O^T^T @ Wp^T      (head-PAIR packed K=128 accumulation)
Host sums the 4 head-group partials per batch and adds bias.
"""

import numpy as np

import concourse.bass as bass
import concourse.tile as tile
from concourse import bacc, library_config, mybir

F32 = mybir.dt.float32
F32R = mybir.dt.float32r
FP8 = mybir.dt.float8e4
DRM = mybir.MatmulPerfMode.DoubleRow
EXP = mybir.ActivationFunctionType.Exp

B, S, C = 2, 2048, 1024
H, D = 16, 64
HPC = 4            # heads per core
NCT = C // 128     # 8 contraction tiles
NPR = NCT // 2     # 4 DoubleRow contraction-tile pairs
MT = S // 128      # 16 key/seq tiles
NCH = S // 512     # 4 query chunks
MM_DT = F32R
EXP_SCALE = 2.0 ** -13   # q,k each carry x32 -> 1024; plus attention 1/8


def build_bass(loop_n=None):
    nc = bacc.Bacc("TRN2", target_bir_lowering=False)

    # packed hi/lo fp8: one DMA fetches both residual halves
    # x8 row ct-block layout: [sc(4)][hi|lo][512 tokens] = 4096 cols
    x8_d = nc.dram_tensor("x8", [C, 2 * S], FP8, kind="ExternalInput")
    # wqk8: [hi|lo][512 outdims] = 1024 cols; wv8: [hi|lo][256] = 512 cols
    wqk8_d = nc.dram_tensor("wqk8", [C, 1024], FP8, kind="ExternalInput")
    wv8_d = nc.dram_tensor("wv8", [C, 512], FP8, kind="ExternalInput")
    wp_d = nc.dram_tensor("wp", [256, C], F32R, kind="ExternalInput")
    y_d = nc.dram_tensor("y", [S, C], F32, kind="ExternalOutput")

    def mm(out, lhsT, rhs, start, stop):
        nc.tensor.matmul(out, lhsT, rhs, start=start, stop=stop)

    def mm_dr(out, lhsT, rhs, start, stop):
        nc.tensor.matmul(out, lhsT, rhs, start=start, stop=stop, perf_mode=DRM)

    import contextlib

    with tile.TileContext(nc) as tc:
        with tc.tile_pool(name="persist", bufs=1) as persist:
            loop_ctx = tc.For_i(0, loop_n, 1) if loop_n else contextlib.nullcontext()
            # qkT f-tiles: 0=q_h0|q_h1, 1=q_h2|q_h3, 2=k_h0|k_h1, 3=k_h2|k_h3
            qk_sb = persist.tile([128, 4 * S], MM_DT, tag="qk")
            # V augmented per key tile: [v_h0|1|v_h1|1|v_h2|1|v_h3|1] = 260 cols
            vaug = persist.tile([128, MT * 260], MM_DT, tag="vaug")
            # Wp rows pair-packed: [:, pr*C:(pr+1)*C] = rows of heads 2pr,2pr+1
            wp_sb = persist.tile([128, 2 * C], MM_DT, tag="wp")
            # head 1's Wp rows again at partition base 0: the tail projection
            # reads head 1 from tmpB (base 0) before its DMA-shift would land
            # (the tail pair runs pr1 first, so norm(3,0) is the last norm)
            wp1_sb = persist.tile([64, C], MM_DT, tag="wp1")
            # O^T/Z pair-packed: onorm[pr] rows 0:64 = head 2pr, 64:128 = 2pr+1
            onorm = [
                persist.tile([128, S], MM_DT, tag=f"onorm{p}", name=f"onorm{p}")
                for p in range(2)
            ]

            with loop_ctx:
              with (
                  tc.tile_pool(name="ph_a", bufs=1) as ph_a,
                  tc.tile_pool(name="pt", bufs=4) as pt_pool,
                  tc.tile_pool(name="small", bufs=2) as small,
                  tc.tile_pool(name="yout", bufs=4) as yout,
              ):
                x8_sb = ph_a.tile([128, NCT * 2 * S], FP8, tag="x8")
                wqk8_sb = ph_a.tile([128, NCT * 1024], FP8, tag="wqk8")
                wv8_sb = ph_a.tile([128, NCT * 512], FP8, tag="wv8")

                # DR pair views: dim1 = ct tile index, hl = hi/lo half
                x8_r = x8_sb.rearrange(
                    "p (ct sc hl s) -> p ct sc hl s", sc=4, hl=2, s=512
                )
                wqk8_r = wqk8_sb.rearrange("p (ct hl m) -> p ct hl m", hl=2, m=512)
                wv8_r = wv8_sb.rearrange("p (ct hl m) -> p ct hl m", hl=2, m=256)

                vones = ph_a.tile([128, 260], F32, tag="vones")
                nc.vector.memset(vones, 1.0)
                # all-ones column block for the tail's PE 1/Z row-broadcast
                # (f32r memset crashes walrus -- init f32, convert via copy)
                ones64 = ph_a.tile([128, 64], MM_DT, tag="ones64")
                nc.vector.tensor_copy(ones64, vones[:, 0:64])

                # tiny warm-up exp so the 1283ns activation-table load runs
                # at t~0 instead of on the critical path of the first S tile
                warm = ph_a.tile([1, 2], F32, tag="warm")
                nc.vector.memset(warm, 0.0)
                nc.scalar.activation(warm, warm, EXP)

                # ---- DMA schedule on three parallel queues ----
                # SP: x8 sc-major (the attention stream consumes keys sc by
                # sc); ACT hwdge: wqk8 then wv8 (issued before the first exp
                # enters ACT's queue); Pool swdge: wp (needed only by proj).
                # ct row-block of x8 in DRAM: [sc][hi|lo][512] = 4096 cols
                for sc in range(4):
                    for ct in range(NCT):
                        nc.sync.dma_start(
                            out=x8_sb[:, ct * 4096 + sc * 1024 : ct * 4096 + (sc + 1) * 1024],
                            in_=x8_d[ct * 128 : (ct + 1) * 128, sc * 1024 : (sc + 1) * 1024],
                        )
                for ct in range(NCT):
                    nc.scalar.dma_start(
                        out=wqk8_sb[:, ct * 1024 : (ct + 1) * 1024],
                        in_=wqk8_d[ct * 128 : (ct + 1) * 128, :],
                    )
                for ct in range(NCT):
                    nc.scalar.dma_start(
                        out=wv8_sb[:, ct * 512 : (ct + 1) * 512],
                        in_=wv8_d[ct * 128 : (ct + 1) * 128, :],
                    )
                for pr in range(2):
                    nc.gpsimd.dma_start(
                        out=wp_sb[:, pr * C : (pr + 1) * C],
                        in_=wp_d[pr * 128 : (pr + 1) * 128, :],
                    )
                nc.gpsimd.dma_start(out=wp1_sb, in_=wp_d[64:128, :])

                # ---- phase A: q,k projections via DoubleRow fp8 ----
                # per (f, sc) psum tile: 12 DR matmuls = 3 products x 4 ct-pairs
                def qk_products(f, sc):
                    """Yield (lhsT, rhs) for the 12 DR matmuls of tile (f, sc):
                    per ct-pair the 3 residual cross products hh, lh, hl."""
                    fs = slice(f * 128, (f + 1) * 128)
                    for p in range(NPR):
                        cs = slice(2 * p, 2 * p + 2)
                        yield wqk8_r[:, cs, 0, fs], x8_r[:, cs, sc, 0, :]
                        yield wqk8_r[:, cs, 1, fs], x8_r[:, cs, sc, 0, :]
                        yield wqk8_r[:, cs, 0, fs], x8_r[:, cs, sc, 1, :]

                def qk_group(f, sc, pool, tag):
                    qps = pool.tile([128, 512], F32, tag=tag, name=f"qps{f}_{sc}")
                    prods = list(qk_products(f, sc))
                    for i, (lt, rh) in enumerate(prods):
                        mm_dr(qps, lt, rh, start=(i == 0), stop=(i == len(prods) - 1))
                    nc.vector.tensor_copy(
                        qk_sb[:, f * S + sc * 512 : f * S + (sc + 1) * 512], qps
                    )

                # ---- V-projection groups, deferred into chunk 0's stream ----
                def v_group(st, ps_v):
                    vps = ps_v.tile([128, 256], F32, tag="vps")
                    sc, to = st // 4, (st % 4) * 128
                    ts = slice(to, to + 128)
                    prods = []
                    for p in range(NPR):
                        cs = slice(2 * p, 2 * p + 2)
                        prods.append((x8_r[:, cs, sc, 0, ts], wv8_r[:, cs, 0, :]))
                        prods.append((x8_r[:, cs, sc, 1, ts], wv8_r[:, cs, 0, :]))
                        prods.append((x8_r[:, cs, sc, 0, ts], wv8_r[:, cs, 1, :]))
                    for i, (lt, rh) in enumerate(prods):
                        mm_dr(vps, lt, rh, start=(i == 0), stop=(i == len(prods) - 1))
                    nc.vector.tensor_copy(
                        vaug[:, st * 260 : (st + 1) * 260].rearrange(
                            "p (h c) -> p h c", c=65
                        )[:, :, 0:64],
                        vps.rearrange("p (h c) -> p h c", c=64),
                    )

                # ---------------- phase B/C: attention ----------------
                with (
                    tc.tile_pool(name="ps_s", bufs=2, space="PSUM") as ps_s,
                    tc.tile_pool(name="ps_o", bufs=2, space="PSUM") as ps_o,
                ):
                    pending_proj = [None]   # chunk index awaiting projection

                    last_tmpB = [None]
                    last_rzps = [None]

                    def tail_norm(pr, ch, oA, oB):
                        """Last-processed unit's normalize: PE row-broadcast of
                        1/Z (213ns) + ACT psum->sbuf copy replace the ~2.5us
                        broadcast DMA, shrinking the exposed tail chain.
                        Emits only DVE/ACT work; PE pieces are returned as
                        thunks for the caller to emit into the tail stream."""
                        rzps = ps_s.tile([128, 1024], F32, tag="sps", name="rzps")
                        last_rzps[0] = rzps
                        on = onorm[pr][:, ch * 512 : (ch + 1) * 512]

                        def chain(oX, half, out_mul, name):
                            rz = small.tile([128, 512], MM_DT, tag="rz", name=f"rz{name}")
                            with nc.allow_low_precision(reason="f32r shares f32 bits"):
                                nc.vector.reciprocal(out=rz[64:65, :], in_=oX[64:65, :])
                            # PE: rzps[0:64, half] = broadcast of rz row (the
                            # caller emits this into the PE stream)
                            def bcast():
                                nc.tensor.matmul(
                                    rzps[0:64, half * 512 : (half + 1) * 512],
                                    ones64[64:65, 0:64],
                                    rz[64:65, :],
                                    start=True,
                                    stop=True,
                                )
                            rzsb = small.tile([128, 512], F32, tag="rzb", name=f"rzsb{name}")
                            def copy():
                                nc.scalar.copy(rzsb[0:64, :], rzps[0:64, half * 512 : (half + 1) * 512])
                            def mul():
                                nc.vector.tensor_mul(out_mul, oX[0:64, :], rzsb[0:64, :])
                            return bcast, copy, mul

                        tmpB = small.tile([64, 512], MM_DT, tag="tmpb", name="tmpB")
                        last_tmpB[0] = tmpB
                        bB, cB, mB = chain(oB, 0, tmpB, "B")
                        bA, cA, mA = chain(oA, 1, on[0:64], "A")
                        return bB, cB, mB, bA, cA, mA

                    def make_norm(pr, ch, oA, oB, skip_shift=False):
                        def bcast(oX, name):
                            # 1/Z broadcast row 64 -> partitions 0:64 via a
                            # stride-0 source DMA (64 descriptors reading the
                            # same SBUF row; engines can't cross partitions)
                            rz = small.tile([128, 512], F32, tag="rz", name=f"rz{name}")
                            nc.vector.reciprocal(out=rz[64:65, :], in_=oX[64:65, :])
                            rzb = small.tile([128, 512], F32, tag="rzb", name=f"rzb{name}")
                            nc.sync.dma_start(
                                out=rzb[0:64, :],
                                in_=rz[64:65, :].unsqueeze(1).to_broadcast((1, 64, 512)),
                            )
                            return rzb

                        def norm():
                            on = onorm[pr][:, ch * 512 : (ch + 1) * 512]
                            # head B first: its chain is longest (normalize at
                            # base 0, then DMA shifts the result into
                            # partitions 64:128 -- DVE can't cross partitions)
                            rzbB = bcast(oB, "B")
                            tmpB = small.tile([64, 512], MM_DT, tag="tmpb", name="tmpB")
                            nc.vector.tensor_mul(tmpB, oB[0:64, :], rzbB[0:64, :])
                            if skip_shift:
                                # tail: the projection reads tmpB directly
                                last_tmpB[0] = tmpB
                            else:
                                nc.sync.dma_start(out=on[64:128], in_=tmpB)
                            # head A: rows 0:64 of onorm[pr], all DVE base 0
                            rzbA = bcast(oA, "A")
                            nc.vector.tensor_mul(on[0:64], oA[0:64, :], rzbA[0:64, :])
                        return norm

                    proj_tiles = {}

                    def proj_start(pch, gi, ps_y, tag="yps"):
                        # pr0 half of group gi: gated only by norm(pch, pr0),
                        # which finished a unit ago -- free filler work for
                        # the unit-start pipeline bubbles
                        st, fc = 4 * pch + gi // 2, gi % 2
                        yps = ps_y.tile([128, 512], F32, tag=tag)
                        proj_tiles[gi] = yps
                        mm(
                            yps,
                            onorm[0][:, st * 128 : (st + 1) * 128],
                            wp_sb[:, fc * 512 : (fc + 1) * 512],
                            start=True,
                            stop=False,
                        )

                    def proj_stop(pch, gi):
                        st, fc = 4 * pch + gi // 2, gi % 2
                        yps = proj_tiles.pop(gi)
                        mm(
                            yps,
                            onorm[1][:, st * 128 : (st + 1) * 128],
                            wp_sb[:, C + fc * 512 : C + (fc + 1) * 512],
                            start=False,
                            stop=True,
                        )
                        ysb = yout.tile([128, 512], F32, tag="ysb")
                        nc.vector.tensor_copy(ysb, yps)
                        nc.sync.dma_start(
                            out=y_d[st * 128 : (st + 1) * 128, fc * 512 : (fc + 1) * 512],
                            in_=ysb,
                        )

                    tail_thunks = [None]

                    def unit(ch, pr, ps_y, v_pool, defer=None, slot=None,
                             defer_pool=None, defer_tag="vps", is_tail=False):
                        """S -> exp -> PV for heads (2pr, 2pr+1), query chunk ch.
                        slot = position within the pair (0/1) for the proj
                        interleave schedule; defaults to pr."""
                        if slot is None:
                            slot = pr
                        if defer_pool is None:
                            defer_pool = v_pool
                        qf, kf = pr, 2 + pr
                        oA = ps_o.tile([128, 512], F32, tag="ops", name="oA")
                        oB = ps_o.tile([128, 512], F32, tag="ops", name="oB")
                        for g in range(8):
                            sA = ps_s.tile([128, 1024], F32, tag="sps", name="sA")
                            sB = ps_s.tile([128, 1024], F32, tag="sps", name="sB")
                            # A-half (mms + exp) emitted fully before B-half:
                            # exp-A's sem wait then can't be coalesced with
                            # the B-mms, which at unit starts still wait on
                            # the previous unit's last exp
                            ptA = pt_pool.tile([128, 1024], MM_DT, tag="pt", name="ptA")
                            ptB = pt_pool.tile([128, 1024], MM_DT, tag="pt", name="ptB")
                            for j in range(2):
                                m = 2 * g + j
                                # two heads row-packed: B in PE rows 64-127,
                                # A in rows 0-63 (base_partition-derived)
                                mm(
                                    sB[:, j * 512 : (j + 1) * 512],
                                    qk_sb[64:128, kf * S + m * 128 : kf * S + (m + 1) * 128],
                                    qk_sb[64:128, qf * S + ch * 512 : qf * S + (ch + 1) * 512],
                                    start=True,
                                    stop=True,
                                )
                            nc.scalar.activation(ptB, sB, EXP, scale=EXP_SCALE)
                            for j in range(2):
                                m = 2 * g + j
                                mm(
                                    sA[:, j * 512 : (j + 1) * 512],
                                    qk_sb[0:64, kf * S + m * 128 : kf * S + (m + 1) * 128],
                                    qk_sb[0:64, qf * S + ch * 512 : qf * S + (ch + 1) * 512],
                                    start=True,
                                    stop=True,
                                )
                            nc.scalar.activation(ptA, sA, EXP, scale=EXP_SCALE)
                            if v_pool is not None and pr == 0:
                                # chunk 0 / pr 0: two V st-groups per g-slot,
                                # just ahead of the PV group that reads them
                                v_group(2 * g, v_pool)
                                v_group(2 * g + 1, v_pool)
                            if pending_proj[0] is not None:
                                # start-halves (ungated) fill unit-start
                                # bubbles; stop-halves wait for norm(pch,pr1)
                                # (~3.4us into this unit) and bank recycling
                                pch = pending_proj[0]
                                if slot == 0:
                                    sched = {
                                        0: (("s", 0), ("s", 1)),
                                        4: (("e", 0), ("e", 1)),
                                        5: (("s", 2), ("s", 3)),
                                        6: (("e", 2), ("e", 3)),
                                        7: (("s", 4), ("s", 5)),
                                    }
                                else:
                                    sched = {
                                        0: (("e", 4), ("e", 5)),
                                        1: (("s", 6), ("s", 7)),
                                        2: (("e", 6), ("e", 7)),
                                    }
                                for kind, gi in sched.get(g, ()):
                                    if kind == "s":
                                        proj_start(pch, gi, ps_y)
                                    else:
                                        proj_stop(pch, gi)
                                if slot == 1 and g == 2:
                                    pending_proj[0] = None
                            if defer is not None:
                                # deferred qk projection tiles: emitted after
                                # the proj schedule so pool-slot WAR waits
                                # only point at earlier instructions
                                for f, sc in defer.get(g, ()):
                                    qk_group(f, sc, defer_pool, defer_tag)
                            hA, hB = 2 * pr, 2 * pr + 1
                            for j in range(2):
                                m = 2 * g + j
                                mm(
                                    oB[0:65, :],
                                    vaug[:, m * 260 + 65 * hB : m * 260 + 65 * hB + 65],
                                    ptB[:, j * 512 : (j + 1) * 512],
                                    start=(m == 0),
                                    stop=(m == MT - 1),
                                )
                                mm(
                                    oA[0:65, :],
                                    vaug[:, m * 260 + 65 * hA : m * 260 + 65 * hA + 65],
                                    ptA[:, j * 512 : (j + 1) * 512],
                                    start=(m == 0),
                                    stop=(m == MT - 1),
                                )
                        # normalize runs off the PE critical path -- emit at
                        # unit end, it overlaps the next unit's S/exp stream
                        if is_tail:
                            tail_thunks[0] = tail_norm(pr, ch, oA, oB)
                        else:
                            make_norm(pr, ch, oA, oB)()

                    # chunk 0: upfront minimum qk tiles, then V-projection and
                    # the remaining qk tiles interleaved into the unit stream
                    with tc.tile_pool(name="ps_v", bufs=2, space="PSUM") as ps_v:
                        qk_group(2, 0, ps_v, "vps")   # k heads 0,1; keys sc0
                        qk_group(0, 0, ps_v, "vps")   # q heads 0,1; chunk 0
                        # vaug ones-row init: after the upfront qk copies in
                        # the DVE queue (so the first S isn't delayed), before
                        # the first v_group copy reads each tile
                        for st in range(MT):
                            nc.vector.tensor_copy(
                                vaug[:, st * 260 : (st + 1) * 260], vones
                            )
                        unit(0, 0, None, ps_v, defer={
                            0: [(2, 1)], 1: [(2, 2)], 2: [(2, 3)],
                            3: [(3, 0)], 4: [(3, 1)], 5: [(3, 2)],
                            6: [(3, 3)], 7: [(1, 0)],
                        })
                        unit(0, 1, None, ps_v, defer={0: [(0, 1)], 4: [(1, 1)]})
                    with tc.tile_pool(name="ps_y", bufs=2, space="PSUM") as ps_y:
                        pending_proj[0] = 0
                        unit(1, 0, ps_y, None, defer={0: [(0, 2)]},
                             defer_pool=ps_y, defer_tag="yps")
                        unit(1, 1, ps_y, None, defer={0: [(1, 2)]},
                             defer_pool=ps_y, defer_tag="yps")
                        pending_proj[0] = 1
                        unit(2, 0, ps_y, None, defer={0: [(0, 3)]},
                             defer_pool=ps_y, defer_tag="yps")
                        unit(2, 1, ps_y, None, defer={0: [(1, 3)]},
                             defer_pool=ps_y, defer_tag="yps")
                        # pair 3 runs pr1 FIRST so the final normalize is
                        # norm(3,0), whose outputs feed only the small K=64
                        # halves of the tail projection
                        pending_proj[0] = 2
                        unit(3, 1, ps_y, None, slot=0)
                        unit(3, 0, ps_y, None, slot=1, is_tail=True)

                        # ---- tail: chunk 3 projection ----
                        # pr1 K=128 halves are ungated (norm(3,1) finished a
                        # unit ago); the pr0 halves read onorm[0][0:64] and
                        # tmpB via wp1, fed by the PE-broadcast norm chain.
                        bB, cB, mB, bA, cA, mA = tail_thunks[0]
                        # bridge matmuls keep the PE p-state warm while the
                        # 1/Z chain (DVE recip -> PE bcast -> ACT copy -> DVE
                        # mul) completes; a cold PE runs 2-4x slower
                        rzps = last_rzps[0]
                        for d in range(8):
                            mm(
                                rzps[:, 0:512],
                                qk_sb[0:64, 0:128],
                                qk_sb[0:64, 0:512],
                                start=True,
                                stop=True,
                            )
                        bB()
                        cB()
                        mB()
                        bA()
                        cA()
                        mA()
                        for st in range(4 * (NCH - 1), 4 * NCH):
                            for fc in range(2):
                                gi = 2 * (st - 4 * (NCH - 1)) + fc
                                if gi % 2 == 0:
                                    yps = ps_y.tile([128, 512], F32, tag="yps")
                                else:
                                    yps = ps_s.tile([128, 512], F32, tag="sps")
                                mm(
                                    yps,
                                    onorm[1][:, st * 128 : (st + 1) * 128],
                                    wp_sb[:, C + fc * 512 : C + (fc + 1) * 512],
                                    start=True,
                                    stop=False,
                                )
                                mm(
                                    yps,
                                    last_tmpB[0][:, (st - 12) * 128 : (st - 11) * 128],
                                    wp1_sb[:, fc * 512 : (fc + 1) * 512],
                                    start=False,
                                    stop=False,
                                )
                                mm(
                                    yps,
                                    onorm[0][0:64, st * 128 : (st + 1) * 128],
                                    wp_sb[0:64, fc * 512 : (fc + 1) * 512],
                                    start=False,
                                    stop=True,
                                )
                                ysb = yout.tile([128, 512], F32, tag="ysb")
                                nc.vector.tensor_copy(ysb, yps)
                                nc.sync.dma_start(
                                    out=y_d[st * 128 : (st + 1) * 128, fc * 512 : (fc + 1) * 512],
                                    in_=ysb,
                                )

    nc.compile()
    return nc


def make_core_inputs(x, Wqkv, Wproj):
    """Per-core input dicts. Core c: batch c//4, heads 4*(c%4) .. 4*(c%4)+3."""
    import ml_dtypes

    e4 = ml_dtypes.float8_e4m3

    def split8(a):
        hi = np.ascontiguousarray(a).astype(e4)
        lo = (a - hi.astype(np.float32)).astype(e4)
        return hi, lo

    def pack_hl(hi, lo, block):
        """[C, M] hi/lo -> [C, 2*M] with [block]-col groups interleaved hi|lo."""
        Cr, M = hi.shape
        out = np.empty((Cr, M // block, 2, block), dtype=e4)
        out[:, :, 0, :] = hi.reshape(Cr, M // block, block)
        out[:, :, 1, :] = lo.reshape(Cr, M // block, block)
        return np.ascontiguousarray(out.reshape(Cr, 2 * M))

    x8s = []
    for b in range(B):
        xt = np.ascontiguousarray(x[b].T).astype(np.float32)
        xh, xl = split8(xt)
        x8s.append(pack_hl(xh, xl, 512))
    in_maps = []
    for core in range(8):
        b, hg = core // 4, core % 4
        heads = [HPC * hg + i for i in range(HPC)]
        rows_q = np.concatenate([Wqkv[D * h : D * (h + 1)] for h in heads])
        rows_k = np.concatenate([Wqkv[C + D * h : C + D * (h + 1)] for h in heads])
        wqk32 = np.ascontiguousarray(
            np.concatenate([rows_q, rows_k]).T, dtype=np.float32
        ) * 32.0
        wqk8 = pack_hl(*split8(wqk32), 512)
        wv32 = np.ascontiguousarray(
            np.concatenate(
                [Wqkv[2 * C + D * h : 2 * C + D * (h + 1)] for h in heads]
            ).T,
            dtype=np.float32,
        ) * 32.0
        wv8 = pack_hl(*split8(wv32), 256)
        wp = np.ascontiguousarray(
            np.concatenate([Wproj[:, D * h : D * (h + 1)] for h in heads], axis=1).T,
            dtype=np.float32,
        ) / 32.0
        in_maps.append({"x8": x8s[b], "wqk8": wqk8, "wv8": wv8, "wp": wp})
    return in_maps


_EXEC_CACHE = {}


def _get_executor():
    """Build + jit the 8-core SPMD executable once per process."""
    if "fn" in _EXEC_CACHE:
        return _EXEC_CACHE
    import jax
    from jax.sharding import Mesh, PartitionSpec
    from jax.experimental.shard_map import shard_map
    from concourse import bass2jax
    from concourse.bass2jax import _bass_exec_p, partition_id_tensor

    nc = build_bass()
    bass2jax.install_neuronx_cc_hook()
    pid = nc.partition_id_tensor.name if nc.partition_id_tensor else None
    in_names, out_names, out_avals = [], [], []
    for alloc in nc.m.functions[0].allocations:
        if not isinstance(alloc, mybir.MemoryLocationSet):
            continue
        name = alloc.memorylocations[0].name
        if alloc.kind == "ExternalInput":
            if name != pid:
                in_names.append(name)
        elif alloc.kind == "ExternalOutput":
            out_names.append(name)
            out_avals.append(
                jax.core.ShapedArray(
                    tuple(alloc.tensor_shape), mybir.dt.np(alloc.dtype)
                )
            )
    n_params = len(in_names)
    all_names = list(in_names) + list(out_names) + ([pid] if pid else [])

    def body(*args):
        *ins, yb = args
        operands = list(ins) + [yb]
        if pid:
            operands.append(partition_id_tensor())
        outs = _bass_exec_p.bind(
            *operands,
            out_avals=tuple(out_avals),
            in_names=tuple(all_names),
            out_names=tuple(out_names),
            lowering_input_output_aliases=(),
            sim_require_finite=True,
            sim_require_nnan=True,
            nc=nc,
        )
        return outs[0]

    mesh = Mesh(np.asarray(jax.devices()[:8]), ("core",))
    fn = jax.jit(
        shard_map(
            body,
            mesh=mesh,
            in_specs=(PartitionSpec("core"),) * (n_params + 1),
            out_specs=PartitionSpec("core"),
            check_rep=False,
        ),
        donate_argnums=(n_params,),
    )
    _EXEC_CACHE.update(fn=fn, in_names=in_names)
    return _EXEC_CACHE


def kernel(x, Wqkv, Wproj, bproj):
    x = np.asarray(x, dtype=np.float32)
    Wqkv = np.asarray(Wqkv, dtype=np.float32)
    Wproj = np.asarray(Wproj, dtype=np.float32)
    bproj = np.asarray(bproj, dtype=np.float32)

    ex = _get_executor()
    in_maps = make_core_inputs(x, Wqkv, Wproj)
    glob_ins = [
        np.concatenate([np.asarray(m[name]) for m in in_maps], axis=0)
        for name in ex["in_names"]
    ]
    y0 = np.zeros((8 * S, C), np.float32)
    out = np.asarray(ex["fn"](*glob_ins, y0))  # [8*S, C]

    y = np.zeros((B, S, C), dtype=np.float32)
    for core in range(8):
        y[core // 4] += out[core * S : (core + 1) * S, :]
    y += bproj
    return y


# revision 3
# speedup vs baseline: 1.0148x; 1.0148x over previous
"""Multi-head attention block (B=2, N=2048, C=1024, H=16, D=64) on 8 TRN2 cores.

Sharding: core c -> batch b = c // 4, head-group hg = c % 4 (4 heads per core).
v2: qkv projections via fp8e4 DoubleRow matmuls (0.5 cyc/row) with host-side
residual splits x ~= xh + xl, W*32 ~= Wh + Wl; three cross products
(Wh.xh + Wh.xl + Wl.xh) recover ~bf16 accuracy (measured 1.7e-3 end-to-end
vs 3.7e-3 for the bf16 baseline) at 3/4 of half the bf16 cycle cost.
Scale bookkeeping: q,k carry x32 each -> exp scale 2^-13 folds the 1/8
attention scale; v carries x32 -> Wproj pre-divided by 32 on host.

Each core:
  qkT,V = W_hg @ x_b^T          (fp8 DoubleRow, 12 DR matmuls per psum tile)
  S^T  = kT^T q-chunks          (fp32r, K=64, two heads row-packed per pass)
  P^T  = exp(S^T * 2^-13)       (ScalarE)
  O^T|Z = [V|1]^T @ P^T         (PSUM-accumulated over key tiles)
  O^T /= Z                      (DVE reciprocal; 1/Z row broadcast via DMA)
  y_partial = O^T^T @ Wp^T      (head-PAIR packed K=128 accumulation)
Host sums the 4 head-group partials per batch and adds bias.
"""

import numpy as np

import concourse.bass as bass
import concourse.tile as tile
from concourse import bacc, library_config, mybir

F32 = mybir.dt.float32
F32R = mybir.dt.float32r
FP8 = mybir.dt.float8e4
DRM = mybir.MatmulPerfMode.DoubleRow
EXP = mybir.ActivationFunctionType.Exp

B, S, C = 2, 2048, 1024
H, D = 16, 64
HPC = 4            # heads per core
NCT = C // 128     # 8 contraction tiles
NPR = NCT // 2     # 4 DoubleRow contraction-tile pairs
MT = S // 128      # 16 key/seq tiles
NCH = S // 512     # 4 query chunks
MM_DT = F32R
EXP_SCALE = 2.0 ** -13   # q,k each carry x32 -> 1024; plus attention 1/8


def build_bass(loop_n=None):
    nc = bacc.Bacc("TRN2", target_bir_lowering=False)

    # packed hi/lo fp8: one DMA fetches both residual halves
    # x8 row ct-block layout: [sc(4)][hi|lo][512 tokens] = 4096 cols
    x8_d = nc.dram_tensor("x8", [C, 2 * S], FP8, kind="ExternalInput")
    # wqk8: [hi|lo][512 outdims] = 1024 cols; wv8: [hi|lo][256] = 512 cols
    wqk8_d = nc.dram_tensor("wqk8", [C, 1024], FP8, kind="ExternalInput")
    wv8_d = nc.dram_tensor("wv8", [C, 512], FP8, kind="ExternalInput")
    wp_d = nc.dram_tensor("wp", [256, C], F32R, kind="ExternalInput")
    y_d = nc.dram_tensor("y", [S, C], F32, kind="ExternalOutput")

    def mm(out, lhsT, rhs, start, stop):
        nc.tensor.matmul(out, lhsT, rhs, start=start, stop=stop)

    def mm_dr(out, lhsT, rhs, start, stop):
        nc.tensor.matmul(out, lhsT, rhs, start=start, stop=stop, perf_mode=DRM)

    import contextlib

    with tile.TileContext(nc) as tc:
        with tc.tile_pool(name="persist", bufs=1) as persist:
            loop_ctx = tc.For_i(0, loop_n, 1) if loop_n else contextlib.nullcontext()
            # qkT f-tiles: 0=q_h0|q_h1, 1=q_h2|q_h3, 2=k_h0|k_h1, 3=k_h2|k_h3
            qk_sb = persist.tile([128, 4 * S], MM_DT, tag="qk")
            # V augmented per key tile: [v_h0|1|v_h1|1|v_h2|1|v_h3|1] = 260 cols
            vaug = persist.tile([128, MT * 260], MM_DT, tag="vaug")
            # Wp rows pair-packed: [:, pr*C:(pr+1)*C] = rows of heads 2pr,2pr+1
            wp_sb = persist.tile([128, 2 * C], MM_DT, tag="wp")
            # head 1's Wp rows again at partition base 0: the tail projection
            # reads head 1 from tmpB (base 0) before its DMA-shift would land
            # (the tail pair runs pr1 first, so norm(3,0) is the last norm)
            wp1_sb = persist.tile([64, C], MM_DT, tag="wp1")
            # O^T/Z pair-packed: onorm[pr] rows 0:64 = head 2pr, 64:128 = 2pr+1
            onorm = [
                persist.tile([128, S], MM_DT, tag=f"onorm{p}", name=f"onorm{p}")
                for p in range(2)
            ]

            with loop_ctx:
              with (
                  tc.tile_pool(name="ph_a", bufs=1) as ph_a,
                  tc.tile_pool(name="pt", bufs=4) as pt_pool,
                  tc.tile_pool(name="small", bufs=2) as small,
                  tc.tile_pool(name="yout", bufs=4) as yout,
              ):
                x8_sb = ph_a.tile([128, NCT * 2 * S], FP8, tag="x8")
                wqk8_sb = ph_a.tile([128, NCT * 1024], FP8, tag="wqk8")
                wv8_sb = ph_a.tile([128, NCT * 512], FP8, tag="wv8")

                # DR pair views: dim1 = ct tile index, hl = hi/lo half
                x8_r = x8_sb.rearrange(
                    "p (ct sc hl s) -> p ct sc hl s", sc=4, hl=2, s=512
                )
                wqk8_r = wqk8_sb.rearrange("p (ct hl m) -> p ct hl m", hl=2, m=512)
                wv8_r = wv8_sb.rearrange("p (ct hl m) -> p ct hl m", hl=2, m=256)

                vones = ph_a.tile([128, 260], F32, tag="vones")
                nc.vector.memset(vones, 1.0)
                # all-ones column block for the tail's PE 1/Z row-broadcast
                # (f32r memset crashes walrus -- init f32, convert via copy)
                ones64 = ph_a.tile([128, 64], MM_DT, tag="ones64")
                nc.vector.tensor_copy(ones64, vones[:, 0:64])

                # ---- DMA schedule on three parallel queues ----
                # SP: x8 sc-major (the attention stream consumes keys sc by
                # sc); ACT hwdge: wqk8 then wv8 (issued before the first exp
                # enters ACT's queue); Pool swdge: wp (needed only by proj).
                # ct row-block of x8 in DRAM: [sc][hi|lo][512] = 4096 cols
                for sc in range(4):
                    for ct in range(NCT):
                        nc.sync.dma_start(
                            out=x8_sb[:, ct * 4096 + sc * 1024 : ct * 4096 + (sc + 1) * 1024],
                            in_=x8_d[ct * 128 : (ct + 1) * 128, sc * 1024 : (sc + 1) * 1024],
                        )
                for ct in range(NCT):
                    nc.scalar.dma_start(
                        out=wqk8_sb[:, ct * 1024 : (ct + 1) * 1024],
                        in_=wqk8_d[ct * 128 : (ct + 1) * 128, :],
                    )
                # warm-up exp after the wqk8 issues: the 1283ns activation-
                # table load runs in ACT's queue slack, not on the critical
                # path of the first wqk8 DMA or the first S tile
                warm = ph_a.tile([1, 2], F32, tag="warm")
                nc.vector.memset(warm, 0.0)
                nc.scalar.activation(warm, warm, EXP)
                for ct in range(NCT):
                    nc.scalar.dma_start(
                        out=wv8_sb[:, ct * 512 : (ct + 1) * 512],
                        in_=wv8_d[ct * 128 : (ct + 1) * 128, :],
                    )
                for pr in range(2):
                    nc.gpsimd.dma_start(
                        out=wp_sb[:, pr * C : (pr + 1) * C],
                        in_=wp_d[pr * 128 : (pr + 1) * 128, :],
                    )
                nc.gpsimd.dma_start(out=wp1_sb, in_=wp_d[64:128, :])

                # ---- phase A: q,k projections via DoubleRow fp8 ----
                # per (f, sc) psum tile: 12 DR matmuls = 3 products x 4 ct-pairs
                def qk_products(f, sc):
                    """Yield (lhsT, rhs) for the 12 DR matmuls of tile (f, sc):
                    per ct-pair the 3 residual cross products hh, lh, hl."""
                    fs = slice(f * 128, (f + 1) * 128)
                    for p in range(NPR):
                        cs = slice(2 * p, 2 * p + 2)
                        yield wqk8_r[:, cs, 0, fs], x8_r[:, cs, sc, 0, :]
                        yield wqk8_r[:, cs, 1, fs], x8_r[:, cs, sc, 0, :]
                        yield wqk8_r[:, cs, 0, fs], x8_r[:, cs, sc, 1, :]

                def qk_group(f, sc, pool, tag):
                    qps = pool.tile([128, 512], F32, tag=tag, name=f"qps{f}_{sc}")
                    prods = list(qk_products(f, sc))
                    for i, (lt, rh) in enumerate(prods):
                        mm_dr(qps, lt, rh, start=(i == 0), stop=(i == len(prods) - 1))
                    nc.vector.tensor_copy(
                        qk_sb[:, f * S + sc * 512 : f * S + (sc + 1) * 512], qps
                    )

                # ---- V-projection groups, deferred into chunk 0's stream ----
                def v_group(st, ps_v):
                    vps = ps_v.tile([128, 256], F32, tag="vps")
                    sc, to = st // 4, (st % 4) * 128
                    ts = slice(to, to + 128)
                    prods = []
                    for p in range(NPR):
                        cs = slice(2 * p, 2 * p + 2)
                        prods.append((x8_r[:, cs, sc, 0, ts], wv8_r[:, cs, 0, :]))
                        prods.append((x8_r[:, cs, sc, 1, ts], wv8_r[:, cs, 0, :]))
                        prods.append((x8_r[:, cs, sc, 0, ts], wv8_r[:, cs, 1, :]))
                    for i, (lt, rh) in enumerate(prods):
                        mm_dr(vps, lt, rh, start=(i == 0), stop=(i == len(prods) - 1))
                    nc.vector.tensor_copy(
                        vaug[:, st * 260 : (st + 1) * 260].rearrange(
                            "p (h c) -> p h c", c=65
                        )[:, :, 0:64],
                        vps.rearrange("p (h c) -> p h c", c=64),
                    )

                # ---------------- phase B/C: attention ----------------
                with (
                    tc.tile_pool(name="ps_s", bufs=2, space="PSUM") as ps_s,
                    tc.tile_pool(name="ps_o", bufs=2, space="PSUM") as ps_o,
                ):
                    pending_proj = [None]   # chunk index awaiting projection

                    last_tmpB = [None]
                    last_rzps = [None]

                    def tail_norm(pr, ch, oA, oB):
                        """Last-processed unit's normalize: PE row-broadcast of
                        1/Z (213ns) + ACT psum->sbuf copy replace the ~2.5us
                        broadcast DMA, shrinking the exposed tail chain.
                        Emits only DVE/ACT work; PE pieces are returned as
                        thunks for the caller to emit into the tail stream."""
                        rzps = ps_s.tile([128, 1024], F32, tag="sps", name="rzps")
                        last_rzps[0] = rzps
                        on = onorm[pr][:, ch * 512 : (ch + 1) * 512]

                        def chain(oX, half, out_mul, name):
                            rz = small.tile([128, 512], MM_DT, tag="rz", name=f"rz{name}")
                            with nc.allow_low_precision(reason="f32r shares f32 bits"):
                                nc.vector.reciprocal(out=rz[64:65, :], in_=oX[64:65, :])
                            # PE: rzps[0:64, half] = broadcast of rz row (the
                            # caller emits this into the PE stream)
                            def bcast():
                                nc.tensor.matmul(
                                    rzps[0:64, half * 512 : (half + 1) * 512],
                                    ones64[64:65, 0:64],
                                    rz[64:65, :],
                                    start=True,
                                    stop=True,
                                )
                            rzsb = small.tile([128, 512], F32, tag="rzb", name=f"rzsb{name}")
                            def copy():
                                nc.scalar.copy(rzsb[0:64, :], rzps[0:64, half * 512 : (half + 1) * 512])
                            def mul():
                                nc.vector.tensor_mul(out_mul, oX[0:64, :], rzsb[0:64, :])
                            return bcast, copy, mul

                        tmpB = small.tile([64, 512], MM_DT, tag="tmpb", name="tmpB")
                        last_tmpB[0] = tmpB
                        bB, cB, mB = chain(oB, 0, tmpB, "B")
                        bA, cA, mA = chain(oA, 1, on[0:64], "A")
                        return bB, cB, mB, bA, cA, mA

                    def make_norm(pr, ch, oA, oB, skip_shift=False):
                        def bcast(rz, name):
                            # 1/Z broadcast row 64 -> partitions 0:64 via a
                            # stride-0 source DMA (64 descriptors reading the
                            # same SBUF row; engines can't cross partitions)
                            rzb = small.tile([128, 512], F32, tag="rzb", name=f"rzb{name}")
                            nc.sync.dma_start(
                                out=rzb[0:64, :],
                                in_=rz[64:65, :].unsqueeze(1).to_broadcast((1, 64, 512)),
                            )
                            return rzb

                        def norm():
                            on = onorm[pr][:, ch * 512 : (ch + 1) * 512]
                            # Free the PSUM accumulators FAST: the recips and
                            # raw O copies are the only readers of oA/oB, so
                            # the next-next unit's first PV matmuls (which
                            # recycle these ps_o slots) unblock ~1us after
                            # the last PV here -- the 1/Z broadcast-DMA chain
                            # (~3.5us) runs asynchronously off the copies.
                            rzB = small.tile([128, 512], F32, tag="rz", name="rzB")
                            nc.vector.reciprocal(out=rzB[64:65, :], in_=oB[64:65, :])
                            orawB = small.tile([64, 512], F32, tag="oraw", bufs=4, name="orawB")
                            nc.vector.tensor_copy(orawB, oB[0:64, :])
                            rzA = small.tile([128, 512], F32, tag="rz", name="rzA")
                            nc.vector.reciprocal(out=rzA[64:65, :], in_=oA[64:65, :])
                            orawA = small.tile([64, 512], F32, tag="oraw", bufs=4, name="orawA")
                            nc.vector.tensor_copy(orawA, oA[0:64, :])
                            # head B first: its chain is longest (normalize at
                            # base 0, then DMA shifts the result into
                            # partitions 64:128 -- DVE can't cross partitions)
                            rzbB = bcast(rzB, "B")
                            tmpB = small.tile([64, 512], MM_DT, tag="tmpb", name="tmpB")
                            nc.vector.tensor_mul(tmpB, orawB, rzbB[0:64, :])
                            if skip_shift:
                                # tail: the projection reads tmpB directly
                                last_tmpB[0] = tmpB
                            else:
                                nc.sync.dma_start(out=on[64:128], in_=tmpB)
                            # head A: rows 0:64 of onorm[pr], all DVE base 0
                            rzbA = bcast(rzA, "A")
                            nc.vector.tensor_mul(on[0:64], orawA, rzbA[0:64, :])
                        return norm

                    proj_tiles = {}

                    def proj_start(pch, gi, ps_y, tag="yps"):
                        # pr0 half of group gi: gated only by norm(pch, pr0),
                        # which finished a unit ago -- free filler work for
                        # the unit-start pipeline bubbles
                        st, fc = 4 * pch + gi // 2, gi % 2
                        yps = ps_y.tile([128, 512], F32, tag=tag)
                        proj_tiles[gi] = yps
                        mm(
                            yps,
                            onorm[0][:, st * 128 : (st + 1) * 128],
                            wp_sb[:, fc * 512 : (fc + 1) * 512],
                            start=True,
                            stop=False,
                        )

                    def proj_stop(pch, gi):
                        st, fc = 4 * pch + gi // 2, gi % 2
                        yps = proj_tiles.pop(gi)
                        mm(
                            yps,
                            onorm[1][:, st * 128 : (st + 1) * 128],
                            wp_sb[:, C + fc * 512 : C + (fc + 1) * 512],
                            start=False,
                            stop=True,
                        )
                        ysb = yout.tile([128, 512], F32, tag="ysb")
                        nc.vector.tensor_copy(ysb, yps)
                        nc.sync.dma_start(
                            out=y_d[st * 128 : (st + 1) * 128, fc * 512 : (fc + 1) * 512],
                            in_=ysb,
                        )

                    tail_thunks = [None]

                    def unit(ch, pr, ps_y, v_pool, defer=None, slot=None,
                             defer_pool=None, defer_tag="vps", is_tail=False):
                        """S -> exp -> PV for heads (2pr, 2pr+1), query chunk ch.
                        slot = position within the pair (0/1) for the proj
                        interleave schedule; defaults to pr."""
                        if slot is None:
                            slot = pr
                        if defer_pool is None:
                            defer_pool = v_pool
                        qf, kf = pr, 2 + pr

                        def emit_fillers(g):
                            if v_pool is not None and pr == 0:
                                # chunk 0 / pr 0: two V st-groups per g-slot,
                                # just ahead of the PV group that reads them
                                v_group(2 * g, v_pool)
                                v_group(2 * g + 1, v_pool)
                            if pending_proj[0] is not None:
                                # start-halves (ungated) fill unit-start
                                # bubbles; stop-halves wait for norm(pch,pr1)
                                # (~3.4us into this unit) and bank recycling
                                pch = pending_proj[0]
                                if slot == 0:
                                    sched = {
                                        0: (("s", 0), ("s", 1)),
                                        4: (("e", 0), ("e", 1)),
                                        5: (("s", 2), ("s", 3)),
                                        6: (("e", 2), ("e", 3)),
                                        7: (("s", 4), ("s", 5)),
                                    }
                                else:
                                    sched = {
                                        0: (("e", 4), ("e", 5)),
                                        1: (("s", 6), ("s", 7)),
                                        2: (("e", 6), ("e", 7)),
                                    }
                                for kind, gi in sched.get(g, ()):
                                    if kind == "s":
                                        proj_start(pch, gi, ps_y)
                                    else:
                                        proj_stop(pch, gi)
                                if slot == 1 and g == 2:
                                    pending_proj[0] = None
                            if defer is not None:
                                # deferred qk projection tiles
                                for f, sc in defer.get(g, ()):
                                    qk_group(f, sc, defer_pool, defer_tag)

                        oA = ps_o.tile([128, 512], F32, tag="ops", name="oA")
                        oB = ps_o.tile([128, 512], F32, tag="ops", name="oB")
                        for g in range(8):
                            # filler work for g==0 is emitted BEFORE the S
                            # matmuls: the new unit's first S group waits on
                            # the previous unit's last exp freeing its PSUM
                            # slot (~600ns ack latency) -- the fillers bridge
                            # that stall with useful matmuls
                            fill_first = g == 0
                            if fill_first:
                                emit_fillers(g)
                            sA = ps_s.tile([128, 1024], F32, tag="sps", name="sA")
                            sB = ps_s.tile([128, 1024], F32, tag="sps", name="sB")
                            # A-half (mms + exp) emitted fully before B-half:
                            # exp-A's sem wait then can't be coalesced with
                            # the B-mms, which at unit starts still wait on
                            # the previous unit's last exp
                            ptA = pt_pool.tile([128, 1024], MM_DT, tag="pt", name="ptA")
                            ptB = pt_pool.tile([128, 1024], MM_DT, tag="pt", name="ptB")
                            for j in range(2):
                                m = 2 * g + j
                                # two heads row-packed: B in PE rows 64-127,
                                # A in rows 0-63 (base_partition-derived)
                                mm(
                                    sB[:, j * 512 : (j + 1) * 512],
                                    qk_sb[64:128, kf * S + m * 128 : kf * S + (m + 1) * 128],
                                    qk_sb[64:128, qf * S + ch * 512 : qf * S + (ch + 1) * 512],
                                    start=True,
                                    stop=True,
                                )
                            nc.scalar.activation(ptB, sB, EXP, scale=EXP_SCALE)
                            if not fill_first:
                                emit_fillers(g)
                            for j in range(2):
                                m = 2 * g + j
                                mm(
                                    sA[:, j * 512 : (j + 1) * 512],
                                    qk_sb[0:64, kf * S + m * 128 : kf * S + (m + 1) * 128],
                                    qk_sb[0:64, qf * S + ch * 512 : qf * S + (ch + 1) * 512],
                                    start=True,
                                    stop=True,
                                )
                            nc.scalar.activation(ptA, sA, EXP, scale=EXP_SCALE)
                            hA, hB = 2 * pr, 2 * pr + 1
                            for j in range(2):
                                m = 2 * g + j
                                mm(
                                    oB[0:65, :],
                                    vaug[:, m * 260 + 65 * hB : m * 260 + 65 * hB + 65],
                                    ptB[:, j * 512 : (j + 1) * 512],
                                    start=(m == 0),
                                    stop=(m == MT - 1),
                                )
                                mm(
                                    oA[0:65, :],
                                    vaug[:, m * 260 + 65 * hA : m * 260 + 65 * hA + 65],
                                    ptA[:, j * 512 : (j + 1) * 512],
                                    start=(m == 0),
                                    stop=(m == MT - 1),
                                )
                        # normalize runs off the PE critical path -- emit at
                        # unit end, it overlaps the next unit's S/exp stream
                        if is_tail:
                            tail_thunks[0] = tail_norm(pr, ch, oA, oB)
                        else:
                            make_norm(pr, ch, oA, oB)()

                    # chunk 0: upfront minimum qk tiles, then V-projection and
                    # the remaining qk tiles interleaved into the unit stream
                    with tc.tile_pool(name="ps_v", bufs=2, space="PSUM") as ps_v:
                        qk_group(2, 0, ps_v, "vps")   # k heads 0,1; keys sc0
                        qk_group(0, 0, ps_v, "vps")   # q heads 0,1; chunk 0
                        # vaug ones-row init: after the upfront qk copies in
                        # the DVE queue (so the first S isn't delayed), before
                        # the first v_group copy reads each tile
                        for st in range(MT):
                            nc.vector.tensor_copy(
                                vaug[:, st * 260 : (st + 1) * 260], vones
                            )
                        unit(0, 0, None, ps_v, defer={
                            0: [(2, 1)], 1: [(2, 2)], 2: [(2, 3)],
                            3: [(3, 0)], 4: [(3, 1)], 5: [(3, 2)],
                            6: [(3, 3)], 7: [(1, 0)],
                        })
                        unit(0, 1, None, ps_v, defer={0: [(0, 1)], 4: [(1, 1)]})
                    with tc.tile_pool(name="ps_y", bufs=2, space="PSUM") as ps_y:
                        pending_proj[0] = 0
                        unit(1, 0, ps_y, None, defer={0: [(0, 2)]},
                             defer_pool=ps_y, defer_tag="yps")
                        unit(1, 1, ps_y, None, defer={0: [(1, 2)]},
                             defer_pool=ps_y, defer_tag="yps")
                        pending_proj[0] = 1
                        unit(2, 0, ps_y, None, defer={0: [(0, 3)]},
                             defer_pool=ps_y, defer_tag="yps")
                        unit(2, 1, ps_y, None, defer={0: [(1, 3)]},
                             defer_pool=ps_y, defer_tag="yps")
                        # pair 3 runs pr1 FIRST so the final normalize is
                        # norm(3,0), whose outputs feed only the small K=64
                        # halves of the tail projection
                        pending_proj[0] = 2
                        unit(3, 1, ps_y, None, slot=0)
                        unit(3, 0, ps_y, None, slot=1, is_tail=True)

                        # ---- tail: chunk 3 projection ----
                        # pr1 K=128 halves are ungated (norm(3,1) finished a
                        # unit ago); the pr0 halves read onorm[0][0:64] and
                        # tmpB via wp1, fed by the PE-broadcast norm chain.
                        bB, cB, mB, bA, cA, mA = tail_thunks[0]
                        bB()
                        cB()
                        mB()
                        bA()
                        cA()
                        mA()
                        for st in range(4 * (NCH - 1), 4 * NCH):
                            for fc in range(2):
                                gi = 2 * (st - 4 * (NCH - 1)) + fc
                                if gi % 2 == 0:
                                    yps = ps_y.tile([128, 512], F32, tag="yps")
                                else:
                                    yps = ps_s.tile([128, 512], F32, tag="sps")
                                mm(
                                    yps,
                                    onorm[1][:, st * 128 : (st + 1) * 128],
                                    wp_sb[:, C + fc * 512 : C + (fc + 1) * 512],
                                    start=True,
                                    stop=False,
                                )
                                mm(
                                    yps,
                                    last_tmpB[0][:, (st - 12) * 128 : (st - 11) * 128],
                                    wp1_sb[:, fc * 512 : (fc + 1) * 512],
                                    start=False,
                                    stop=False,
                                )
                                mm(
                                    yps,
                                    onorm[0][0:64, st * 128 : (st + 1) * 128],
                                    wp_sb[0:64, fc * 512 : (fc + 1) * 512],
                                    start=False,
                                    stop=True,
                                )
                                ysb = yout.tile([128, 512], F32, tag="ysb")
                                # drain on two engines: ACT is idle after the
                                # last exp, so alternating halves the tail's
                                # psum->sbuf copy latency; the y DMAs issue
                                # round-robin over three queues (a single
                                # queue serializes 8 x 790ns of descriptor
                                # generation after the last matmul)
                                if gi % 2 == 0:
                                    nc.scalar.copy(ysb, yps)
                                else:
                                    nc.vector.tensor_copy(ysb, yps)
                                dq = (nc.sync, nc.scalar, nc.gpsimd)[gi % 3]
                                dq.dma_start(
                                    out=y_d[st * 128 : (st + 1) * 128, fc * 512 : (fc + 1) * 512],
                                    in_=ysb,
                                )

    nc.compile()
    return nc


def make_core_inputs(x, Wqkv, Wproj):
    """Per-core input dicts. Core c: batch c//4, heads 4*(c%4) .. 4*(c%4)+3."""
    import ml_dtypes

    e4 = ml_dtypes.float8_e4m3

    def split8(a):
        hi = np.ascontiguousarray(a).astype(e4)
        lo = (a - hi.astype(np.float32)).astype(e4)
        return hi, lo

    def pack_hl(hi, lo, block):
        """[C, M] hi/lo -> [C, 2*M] with [block]-col groups interleaved hi|lo."""
        Cr, M = hi.shape
        out = np.empty((Cr, M // block, 2, block), dtype=e4)
        out[:, :, 0, :] = hi.reshape(Cr, M // block, block)
        out[:, :, 1, :] = lo.reshape(Cr, M // block, block)
        return np.ascontiguousarray(out.reshape(Cr, 2 * M))

    x8s = []
    for b in range(B):
        xt = np.ascontiguousarray(x[b].T).astype(np.float32)
        xh, xl = split8(xt)
        x8s.append(pack_hl(xh, xl, 512))
    in_maps = []
    for core in range(8):
        b, hg = core // 4, core % 4
        heads = [HPC * hg + i for i in range(HPC)]
        rows_q = np.concatenate([Wqkv[D * h : D * (h + 1)] for h in heads])
        rows_k = np.concatenate([Wqkv[C + D * h : C + D * (h + 1)] for h in heads])
        wqk32 = np.ascontiguousarray(
            np.concatenate([rows_q, rows_k]).T, dtype=np.float32
        ) * 32.0
        wqk8 = pack_hl(*split8(wqk32), 512)
        wv32 = np.ascontiguousarray(
            np.concatenate(
                [Wqkv[2 * C + D * h : 2 * C + D * (h + 1)] for h in heads]
            ).T,
            dtype=np.float32,
        ) * 32.0
        wv8 = pack_hl(*split8(wv32), 256)
        wp = np.ascontiguousarray(
            np.concatenate([Wproj[:, D * h : D * (h + 1)] for h in heads], axis=1).T,
            dtype=np.float32,
        ) / 32.0
        in_maps.append({"x8": x8s[b], "wqk8": wqk8, "wv8": wv8, "wp": wp})
    return in_maps


_EXEC_CACHE = {}


def _get_executor():
    """Build + jit the 8-core SPMD executable once per process."""
    if "fn" in _EXEC_CACHE:
        return _EXEC_CACHE
    import jax
    from jax.sharding import Mesh, PartitionSpec
    from jax.experimental.shard_map import shard_map
    from concourse import bass2jax
    from concourse.bass2jax import _bass_exec_p, partition_id_tensor

    nc = build_bass()
    bass2jax.install_neuronx_cc_hook()
    pid = nc.partition_id_tensor.name if nc.partition_id_tensor else None
    in_names, out_names, out_avals = [], [], []
    for alloc in nc.m.functions[0].allocations:
        if not isinstance(alloc, mybir.MemoryLocationSet):
            continue
        name = alloc.memorylocations[0].name
        if alloc.kind == "ExternalInput":
            if name != pid:
                in_names.append(name)
        elif alloc.kind == "ExternalOutput":
            out_names.append(name)
            out_avals.append(
                jax.core.ShapedArray(
                    tuple(alloc.tensor_shape), mybir.dt.np(alloc.dtype)
                )
            )
    n_params = len(in_names)
    all_names = list(in_names) + list(out_names) + ([pid] if pid else [])

    def body(*args):
        *ins, yb = args
        operands = list(ins) + [yb]
        if pid:
            operands.append(partition_id_tensor())
        outs = _bass_exec_p.bind(
            *operands,
            out_avals=tuple(out_avals),
            in_names=tuple(all_names),
            out_names=tuple(out_names),
            lowering_input_output_aliases=(),
            sim_require_finite=True,
            sim_require_nnan=True,
            nc=nc,
        )
        return outs[0]

    mesh = Mesh(np.asarray(jax.devices()[:8]), ("core",))
    fn = jax.jit(
        shard_map(
            body,
            mesh=mesh,
            in_specs=(PartitionSpec("core"),) * (n_params + 1),
            out_specs=PartitionSpec("core"),
            check_rep=False,
        ),
        donate_argnums=(n_params,),
    )
    _EXEC_CACHE.update(fn=fn, in_names=in_names)
    return _EXEC_CACHE


def kernel(x, Wqkv, Wproj, bproj):
    x = np.asarray(x, dtype=np.float32)
    Wqkv = np.asarray(Wqkv, dtype=np.float32)
    Wproj = np.asarray(Wproj, dtype=np.float32)
    bproj = np.asarray(bproj, dtype=np.float32)

    ex = _get_executor()
    in_maps = make_core_inputs(x, Wqkv, Wproj)
    glob_ins = [
        np.concatenate([np.asarray(m[name]) for m in in_maps], axis=0)
        for name in ex["in_names"]
    ]
    y0 = np.zeros((8 * S, C), np.float32)
    out = np.asarray(ex["fn"](*glob_ins, y0))  # [8*S, C]

    y = np.zeros((B, S, C), dtype=np.float32)
    for core in range(8):
        y[core // 4] += out[core * S : (core + 1) * S, :]
    y += bproj
    return y
